# revision 28
# baseline (speedup 1.0000x reference)
"""Multi-scale stereo cost-volume kernel for 8 Trainium2 NeuronCores.

For each scale i the reference computes
    cost[n, j, h, x] = sum_c l2n(left)[n,c,h,x] * l2n(right)[n,c,h,x-j]
(zero where x < j), with D_i = 128 >> i disparities.

Device strategy (data-parallel over H, 8 cores):
  - Per group of R h-rows (R = 128 // chunk so K = R*C = 128): load
    left/right as [K=(hb,c), W] SBUF tiles, compute per-column L2 norms
    with a block-diagonal-ones matvec on the PE, rsqrt via DVE
    reciprocal + ACT sqrt, broadcast the [R, W] inverse norms back to
    [K, W] with a second ones matmul, and scale.
  - Band matmul per x-chunk: block-diagonal stationary built from the
    normalized left tile, moving operand a [K, B] slice of the
    zero-left-padded right tile, so psum[(hb,x'), q] holds the dot for
    x = x0+x', y = x0+x'+ (q - x') - (D-1). Only the B = chunk+D-1 wide
    band ever gets computed.
  - The psum band is DMAed verbatim to DRAM. The diagonal gather, the
    j-reversal, and the layout transpose are all pure re-indexing, done
    on the host with numpy stride tricks for free.
"""

import sys

import numpy as np

if "/opt/trn_rl_repo" not in sys.path:
    sys.path.insert(0, "/opt/trn_rl_repo")

N_CORES = 8

# C, H, W: per-scale input dims (N=2 batch). D: disparities. R: h-rows
# packed per matmul so the contraction dim R*C fills the 128-wide PE.
SCALES = [
    dict(C=32, H=256, W=512, D=128, R=4),
    dict(C=64, H=128, W=256, D=64, R=2),
    dict(C=96, H=64, W=128, D=32, R=1),
]

LAST = {"exec_time_ns": None, "results": None}
_NC_CACHE = {}


def _derived(s):
    C, H, W, D, R = s["C"], s["H"], s["W"], s["D"], s["R"]
    Hl = H // N_CORES
    K = R * C
    cch = 128 // R
    B = cch + D - 1
    G = 2 * Hl // R
    XC = W // cch
    return C, H, W, D, R, Hl, K, cch, B, G, XC


def _emit_scale(nc, tc, ctx, si, s, l_dram, r_dram, out_dram):
    import concourse.bass as bass
    from concourse import mybir

    f32 = mybir.dt.float32
    C, H, W, D, R, Hl, K, cch, B, G, XC = _derived(s)

    io_pool = ctx.enter_context(tc.tile_pool(name=f"io{si}", bufs=3))
    sq_pool = ctx.enter_context(tc.tile_pool(name=f"sq{si}", bufs=2))
    small_pool = ctx.enter_context(tc.tile_pool(name=f"small{si}", bufs=2))
    ln_pool = ctx.enter_context(tc.tile_pool(name=f"ln{si}", bufs=2))
    pers_pool = ctx.enter_context(tc.tile_pool(name=f"pers{si}", bufs=1))
    rpad_pool = ctx.enter_context(tc.tile_pool(name=f"rpad{si}", bufs=2))
    # PSUM budget: 8 banks. npl+npr (1 each) + bcl+bcr (1 each) + band (4).
    npsum_pool = ctx.enter_context(
        tc.tile_pool(name=f"npsum{si}", bufs=1, space="PSUM")
    )
    bc_pool = ctx.enter_context(tc.tile_pool(name=f"bc{si}", bufs=1, space="PSUM"))
    band_pool = ctx.enter_context(tc.tile_pool(name=f"band{si}", bufs=4, space="PSUM"))
    bout_pool = ctx.enter_context(tc.tile_pool(name=f"bout{si}", bufs=4))

    # ones_bd[(hb,c), m] = 1 iff m == hb: per-h-row column sum of squares.
    # ones_sel[hb, m] = 1 iff m in hb's partition block: broadcast [R,W]->[K,W].
    ones_bd_np = np.zeros((K, R), np.float32)
    ones_sel_np = np.zeros((R, K), np.float32)
    for hb in range(R):
        ones_bd_np[hb * C : (hb + 1) * C, hb] = 1.0
        ones_sel_np[hb, hb * C : (hb + 1) * C] = 1.0
    ones_bd_dram = nc.inline_tensor(ones_bd_np, name=f"ones_bd{si}")
    ones_sel_dram = nc.inline_tensor(ones_sel_np, name=f"ones_sel{si}")
    ones_bd = pers_pool.tile([K, R], f32, tag="ones_bd")
    ones_sel = pers_pool.tile([R, K], f32, tag="ones_sel")
    nc.sync.dma_start(ones_bd[:], ones_bd_dram[:])
    nc.sync.dma_start(ones_sel[:], ones_sel_dram[:])

    # Persistent padded right tiles; left D-1 columns stay zero so the
    # band matmul reads exact zeros for y < 0 (which become the x<j mask).
    rpads = []
    for b in range(2):
        t = rpad_pool.tile([K, D - 1 + W], f32, tag=f"rp{b}")
        nc.gpsimd.memset(t[:, 0 : D - 1], 0.0)
        rpads.append(t)

    # Block-diagonal stationary strips: strip[:, xc*128:(xc+1)*128] is the
    # [K, 128] stationary for chunk xc (block hb of chunk xc lives at
    # partitions [hb*C,(hb+1)*C), cols xc*128 + [hb*cch,(hb+1)*cch)).
    # Zeros off the diagonal blocks are set once and never overwritten.
    strips = []
    if R > 1:
        bd_pool = ctx.enter_context(tc.tile_pool(name=f"bd{si}", bufs=2))
        for b in range(2):
            t = bd_pool.tile([K, XC * 128], f32, tag=f"bd{b}")
            nc.gpsimd.memset(t[:], 0.0)
            strips.append(t)

    CHW = C * Hl * W
    HW = Hl * W
    ci = 0
    for g in range(G):
        nh0 = g * R
        n = nh0 // Hl
        h0 = nh0 % Hl
        off = n * CHW + h0 * W

        l_raw = io_pool.tile([K, W], f32, tag="lraw")
        r_raw = io_pool.tile([K, W], f32, tag="rraw")
        src_l = bass.AP(l_dram, off, [[W, R], [HW, C], [1, W]])
        src_r = bass.AP(r_dram, off, [[W, R], [HW, C], [1, W]])
        nc.gpsimd.dma_start(l_raw[:], src_l)
        nc.gpsimd.dma_start(r_raw[:], src_r)

        sq_l = sq_pool.tile([K, W], f32, tag="sql")
        sq_r = sq_pool.tile([K, W], f32, tag="sqr")
        nc.vector.tensor_mul(sq_l[:], l_raw[:], l_raw[:])
        nc.vector.tensor_mul(sq_r[:], r_raw[:], r_raw[:])

        np_l = npsum_pool.tile([R, W], f32, tag="npl")
        np_r = npsum_pool.tile([R, W], f32, tag="npr")
        nc.tensor.matmul(np_l[:], ones_bd[:], sq_l[:])
        nc.tensor.matmul(np_r[:], ones_bd[:], sq_r[:])

        rec_l = small_pool.tile([R, W], f32, tag="recl")
        rec_r = small_pool.tile([R, W], f32, tag="recr")
        nc.vector.reciprocal(rec_l[:], np_l[:])
        nc.vector.reciprocal(rec_r[:], np_r[:])
        inv_l = small_pool.tile([R, W], f32, tag="invl")
        inv_r = small_pool.tile([R, W], f32, tag="invr")
        nc.scalar.sqrt(inv_l[:], rec_l[:])
        nc.scalar.sqrt(inv_r[:], rec_r[:])

        bc_l = bc_pool.tile([K, W], f32, tag="bcl")
        bc_r = bc_pool.tile([K, W], f32, tag="bcr")
        nc.tensor.matmul(bc_l[:], ones_sel[:], inv_l[:])
        nc.tensor.matmul(bc_r[:], ones_sel[:], inv_r[:])

        rp = rpads[g % 2]
        nc.vector.tensor_mul(rp[:, D - 1 :], r_raw[:], bc_r[:])

        strip = None
        l_n = None
        if R > 1:
            # Normalize-multiply straight into the strip's diagonal blocks:
            # per hb one DVE mul whose output AP scatters the [C, W] product
            # across the XC chunk positions (128-col steps). No DMAs.
            strip = strips[g % 2]
            for hb in range(R):
                dst0 = strip[hb * C : (hb + 1) * C, hb * cch : hb * cch + cch]
                pstride = int(dst0.ap[0][0])
                dst = bass.AP(
                    dst0.tensor, dst0.offset, [[pstride, C], [128, XC], [1, cch]]
                )
                lr0 = l_raw[hb * C : (hb + 1) * C, :]
                lr = bass.AP(lr0.tensor, lr0.offset, [[W, C], [cch, XC], [1, cch]])
                bc0 = bc_l[hb * C : (hb + 1) * C, :]
                bcs = int(bc0.ap[0][0])
                bc = bass.AP(bc0.tensor, bc0.offset, [[bcs, C], [cch, XC], [1, cch]])
                nc.vector.tensor_mul(dst, lr, bc)
        else:
            l_n = ln_pool.tile([K, W], f32, tag="ln")
            nc.vector.tensor_mul(l_n[:], l_raw[:], bc_l[:])

        QB = min(4, XC)
        band_sb = None
        for xc in range(XC):
            x0 = xc * cch
            if R > 1:
                stat = strip[:, xc * 128 : (xc + 1) * 128]
            else:
                stat = l_n[:]
            band = band_pool.tile([128, B], f32, tag="band")
            nc.tensor.matmul(band[:], stat, rp[:, x0 : x0 + B])
            slot = ci % QB
            if slot == 0:
                band_sb = bout_pool.tile([128, QB * B], f32, tag="bout")
            dst = band_sb[:, slot * B : (slot + 1) * B]
            if ci % 2 == 0:
                nc.scalar.copy(dst, band[:])
            else:
                nc.vector.tensor_copy(dst, band[:])
            if slot == QB - 1:
                nc.sync.dma_start(out_dram[ci // QB], band_sb[:])
            ci += 1


def _build_nc():
    if "nc" in _NC_CACHE:
        return _NC_CACHE["nc"]
    from contextlib import ExitStack

    import concourse.tile as tile
    from concourse import bacc, mybir

    f32 = mybir.dt.float32
    nc = bacc.Bacc("TRN2", target_bir_lowering=False, debug=False)

    tensors = []
    for i, s in enumerate(SCALES):
        C, H, W, D, R, Hl, K, cch, B, G, XC = _derived(s)
        l = nc.dram_tensor(f"left{i}", [2, C, Hl, W], f32, kind="ExternalInput")
        r = nc.dram_tensor(f"right{i}", [2, C, Hl, W], f32, kind="ExternalInput")
        QB = min(4, XC)
        o = nc.dram_tensor(
            f"out{i}", [G * XC // QB, 128, QB * B], f32, kind="ExternalOutput"
        )
        tensors.append((l, r, o))

    with tile.TileContext(nc) as tc:
        for i, s in enumerate(SCALES):
            l, r, o = tensors[i]
            with ExitStack() as ctx:
                _emit_scale(nc, tc, ctx, i, s, l, r, o)
    nc.compile()
    _NC_CACHE["nc"] = nc
    return nc


def _assemble(arr, s):
    """[G*XC/QB, 128, QB*B] batched bands -> [2, D, Hl, W] block for one core."""
    C, H, W, D, R, Hl, K, cch, B, G, XC = _derived(s)
    QB = min(4, XC)
    # unpack store batches: [NS, 128, QB, B] -> chunk-major [G*XC, 128, B]
    arr = np.ascontiguousarray(arr).reshape(G, XC // QB, 128, QB, B)
    arr = arr.transpose(0, 1, 3, 2, 4)
    arr = np.ascontiguousarray(arr).reshape(G, XC, R, cch, B)
    e = arr.strides[-1]
    ext = np.lib.stride_tricks.as_strided(
        arr,
        shape=(G, XC, R, cch, D),
        strides=(
            arr.strides[0],
            arr.strides[1],
            arr.strides[2],
            arr.strides[3] + e,  # x' advances one row AND one column: diagonal
            e,
        ),
    )
    # ext[g, xc, hb, x', v] = cost[n, D-1-v, h_local, xc*cch + x']
    out = ext.transpose(0, 2, 4, 1, 3).reshape(2, Hl, D, W)
    out = out.transpose(0, 2, 1, 3)[:, ::-1, :, :]
    return np.ascontiguousarray(out)


def _get_runner():
    """Build (once) a reusable jitted SPMD runner over the 8 cores.

    Mirrors concourse.bass2jax.run_bass_via_pjrt, but: (a) the jitted
    callable is cached so repeat calls don't re-trace/re-compile, and
    (b) the NEFF's output buffers are jnp.zeros created inside the jit
    (every output element is written by the kernel, so their initial
    value is irrelevant) — no per-call host->device output transfer.
    """
    if "runner" in _NC_CACHE:
        return _NC_CACHE["runner"]

    import jax
    import jax.numpy as jnp
    from jax.sharding import Mesh, PartitionSpec
    from jax.experimental.shard_map import shard_map

    from concourse import bass2jax, mybir

    nc = _build_nc()
    bass2jax.install_neuronx_cc_hook()

    partition_name = nc.partition_id_tensor.name if nc.partition_id_tensor else None
    in_names, out_names, out_avals = [], [], []
    for alloc in nc.m.functions[0].allocations:
        if not isinstance(alloc, mybir.MemoryLocationSet):
            continue
        name = alloc.memorylocations[0].name
        if alloc.kind == "ExternalInput":
            if name != partition_name:
                in_names.append(name)
        elif alloc.kind == "ExternalOutput":
            shape = tuple(alloc.tensor_shape)
            dtype = mybir.dt.np(alloc.dtype)
            out_avals.append(jax.core.ShapedArray(shape, dtype))
            out_names.append(name)
    n_params = len(in_names)
    all_in_names = list(in_names) + list(out_names)
    if partition_name is not None:
        all_in_names.append(partition_name)

    def _body(*args):
        operands = list(args)
        if partition_name is not None:
            operands.append(bass2jax.partition_id_tensor())
        outs = bass2jax._bass_exec_p.bind(
            *operands,
            out_avals=tuple(out_avals),
            in_names=tuple(all_in_names),
            out_names=tuple(out_names),
            lowering_input_output_aliases=(),
            sim_require_finite=True,
            sim_require_nnan=True,
            nc=nc,
        )
        return tuple(outs)

    devices = jax.devices()[:N_CORES]
    mesh = Mesh(np.asarray(devices), ("core",))
    sharded = jax.jit(
        shard_map(
            _body,
            mesh=mesh,
            in_specs=(PartitionSpec("core"),) * (n_params + len(out_names)),
            out_specs=(PartitionSpec("core"),) * len(out_names),
            check_rep=False,
        ),
        keep_unused=True,
    )
    runner = dict(
        nc=nc,
        sharded=sharded,
        in_names=in_names,
        out_names=out_names,
        out_avals=out_avals,
        mesh=mesh,
    )
    _NC_CACHE["runner"] = runner
    return runner


def _shard_inputs(lefts, rights):
    """Full inputs -> concat-along-axis0 per-tensor arrays for shard_map."""
    concat = {}
    for i, s in enumerate(SCALES):
        Hl = s["H"] // N_CORES
        for nm, src in ((f"left{i}", lefts[i]), (f"right{i}", rights[i])):
            parts = [
                np.ascontiguousarray(src[:, :, k * Hl : (k + 1) * Hl, :], np.float32)
                for k in range(N_CORES)
            ]
            concat[nm] = np.concatenate(parts, axis=0)
    return concat


def _zero_args(r):
    return [
        np.zeros((N_CORES * a.shape[0], *a.shape[1:]), a.dtype)
        for a in r["out_avals"]
    ]


def _run(concat_inputs):
    r = _get_runner()
    args = [concat_inputs[nm] for nm in r["in_names"]] + _zero_args(r)
    out_arrs = r["sharded"](*args)
    res = []
    for i, nm in enumerate(r["out_names"]):
        a = np.asarray(out_arrs[i])
        res.append(a.reshape(N_CORES, *r["out_avals"][i].shape))
    return dict(zip(r["out_names"], res))


def kernel(left0, right0, left1, right1, left2, right2, max_disparity):
    lefts = [np.asarray(left0), np.asarray(left1), np.asarray(left2)]
    rights = [np.asarray(right0), np.asarray(right1), np.asarray(right2)]

    concat_inputs = _shard_inputs(lefts, rights)
    res = _run(concat_inputs)

    outs = []
    for i, s in enumerate(SCALES):
        blocks = [_assemble(res[f"out{i}"][k], s) for k in range(N_CORES)]
        outs.append(np.concatenate(blocks, axis=2))
    return tuple(outs)


def bench(iters=5):
    """Time repeated device executions with device-resident inputs."""
    import time

    import jax

    rng = np.random.default_rng(0)
    lefts, rights = [], []
    for s in SCALES:
        lefts.append(rng.standard_normal((2, s["C"], s["H"], s["W"])).astype(np.float32))
        rights.append(rng.standard_normal((2, s["C"], s["H"], s["W"])).astype(np.float32))
    concat_inputs = _shard_inputs(lefts, rights)
    r = _get_runner()
    from jax.sharding import NamedSharding, PartitionSpec

    sh = NamedSharding(r["mesh"], PartitionSpec("core"))
    args = [
        jax.device_put(a, sh)
        for a in [concat_inputs[nm] for nm in r["in_names"]] + _zero_args(r)
    ]
    times = []
    for it in range(iters + 1):
        t0 = time.perf_counter()
        out = r["sharded"](*args)
        jax.block_until_ready(out)
        dt = time.perf_counter() - t0
        if it > 0:
            times.append(dt)
    return times


# revision 33
# speedup vs baseline: 1.3203x; 1.3203x over previous
"""Multi-scale stereo cost-volume kernel for 8 Trainium2 NeuronCores.

For each scale i the reference computes
    cost[n, j, h, x] = sum_c l2n(left)[n,c,h,x] * l2n(right)[n,c,h,x-j]
(zero where x < j), with D_i = 128 >> i disparities.

Device strategy (data-parallel over H, 8 cores):
  - Per group of R h-rows (R = 128 // chunk so K = R*C = 128): load
    left/right as [K=(hb,c), W] SBUF tiles, compute per-column L2 norms
    with a block-diagonal-ones matvec on the PE, rsqrt via DVE
    reciprocal + ACT sqrt, broadcast the [R, W] inverse norms back to
    [K, W] with a second ones matmul, and scale.
  - Band matmul per x-chunk: block-diagonal stationary built from the
    normalized left tile, moving operand a [K, B] slice of the
    zero-left-padded right tile, so psum[(hb,x'), q] holds the dot for
    x = x0+x', y = x0+x'+ (q - x') - (D-1). Only the B = chunk+D-1 wide
    band ever gets computed.
  - The psum band is DMAed verbatim to DRAM. The diagonal gather, the
    j-reversal, and the layout transpose are all pure re-indexing, done
    on the host with numpy stride tricks for free.
"""

import sys

import numpy as np

if "/opt/trn_rl_repo" not in sys.path:
    sys.path.insert(0, "/opt/trn_rl_repo")

N_CORES = 8

# C, H, W: per-scale input dims (N=2 batch). D: disparities. R: h-rows
# packed per matmul so the contraction dim R*C fills the 128-wide PE.
SCALES = [
    dict(C=32, H=256, W=512, D=128, R=4),
    dict(C=64, H=128, W=256, D=64, R=2),
    dict(C=96, H=64, W=128, D=32, R=1),
]

LAST = {"exec_time_ns": None, "results": None}
_NC_CACHE = {}


def _derived(s):
    C, H, W, D, R = s["C"], s["H"], s["W"], s["D"], s["R"]
    Hl = H // N_CORES
    K = R * C
    cch = 128 // R
    B = cch + D - 1
    G = 2 * Hl // R
    XC = W // cch
    return C, H, W, D, R, Hl, K, cch, B, G, XC


def _emit_scale(nc, tc, pools, si, s, l_dram, r_dram, out_dram):
    """Pools are shared across scales (tags sized to the max tile) so no
    pool-scope boundary forces the scheduler to serialize scale0 -> 1 -> 2;
    later scales' work fills earlier scales' pipeline gaps."""
    import concourse.bass as bass
    from concourse import mybir

    f32 = mybir.dt.float32
    C, H, W, D, R, Hl, K, cch, B, G, XC = _derived(s)

    io_pool = pools["io"]
    sq_pool = pools["sq"]
    small_pool = pools["small"]
    ln_pool = pools["ln"]
    pers_pool = pools["pers"]
    rpad_pool = pools["rpad"]
    npsum_pool = pools["npsum"]
    bc_pool = pools["bc"]
    band_pool = pools["band"]
    bout_pool = pools["bout"]

    # ones_bd[(hb,c), m] = 1 iff m == hb: per-h-row column sum of squares.
    # ones_sel[hb, m] = 1 iff m in hb's partition block: broadcast [R,W]->[K,W].
    ones_bd_np = np.zeros((K, R), np.float32)
    ones_sel_np = np.zeros((R, K), np.float32)
    for hb in range(R):
        ones_bd_np[hb * C : (hb + 1) * C, hb] = 1.0
        ones_sel_np[hb, hb * C : (hb + 1) * C] = 1.0
    ones_bd_dram = nc.inline_tensor(ones_bd_np, name=f"ones_bd{si}")
    ones_sel_dram = nc.inline_tensor(ones_sel_np, name=f"ones_sel{si}")
    ones_bd = pers_pool.tile([K, R], f32, tag=f"ones_bd{si}")
    ones_sel = pers_pool.tile([R, K], f32, tag=f"ones_sel{si}")
    nc.sync.dma_start(ones_bd[:], ones_bd_dram[:])
    nc.sync.dma_start(ones_sel[:], ones_sel_dram[:])

    # Persistent padded right tiles; left D-1 columns stay zero so the
    # band matmul reads exact zeros for y < 0 (which become the x<j mask).
    rpads = []
    for b in range(2):
        t = rpad_pool.tile([K, D - 1 + W], f32, tag=f"rp{si}_{b}")
        nc.gpsimd.memset(t[:, 0 : D - 1], 0.0)
        rpads.append(t)

    # Block-diagonal stationary strips: strip[:, xc*128:(xc+1)*128] is the
    # [K, 128] stationary for chunk xc (block hb of chunk xc lives at
    # partitions [hb*C,(hb+1)*C), cols xc*128 + [hb*cch,(hb+1)*cch)).
    # Zeros off the diagonal blocks are set once and never overwritten.
    strips = []
    if R > 1:
        for b in range(2):
            t = pools["bd"].tile([K, XC * 128], f32, tag=f"bd{si}_{b}")
            nc.gpsimd.memset(t[:], 0.0)
            strips.append(t)

    CHW = C * Hl * W
    HW = Hl * W
    ci = 0
    for g in range(G):
        nh0 = g * R
        n = nh0 // Hl
        h0 = nh0 % Hl
        off = n * CHW + h0 * W

        l_raw = io_pool.tile([K, W], f32, tag="lraw")
        r_raw = io_pool.tile([K, W], f32, tag="rraw")
        src_l = bass.AP(l_dram, off, [[W, R], [HW, C], [1, W]])
        src_r = bass.AP(r_dram, off, [[W, R], [HW, C], [1, W]])
        nc.gpsimd.dma_start(l_raw[:], src_l)
        nc.gpsimd.dma_start(r_raw[:], src_r)

        sq_l = sq_pool.tile([K, W], f32, tag="sql")
        sq_r = sq_pool.tile([K, W], f32, tag="sqr")
        nc.vector.tensor_mul(sq_l[:], l_raw[:], l_raw[:])
        nc.vector.tensor_mul(sq_r[:], r_raw[:], r_raw[:])

        np_l = npsum_pool.tile([R, W], f32, tag="npl")
        np_r = npsum_pool.tile([R, W], f32, tag="npr")
        nc.tensor.matmul(np_l[:], ones_bd[:], sq_l[:])
        nc.tensor.matmul(np_r[:], ones_bd[:], sq_r[:])

        rec_l = small_pool.tile([R, W], f32, tag="recl")
        rec_r = small_pool.tile([R, W], f32, tag="recr")
        nc.vector.reciprocal(rec_l[:], np_l[:])
        nc.vector.reciprocal(rec_r[:], np_r[:])
        inv_l = small_pool.tile([R, W], f32, tag="invl")
        inv_r = small_pool.tile([R, W], f32, tag="invr")
        nc.scalar.sqrt(inv_l[:], rec_l[:])
        nc.scalar.sqrt(inv_r[:], rec_r[:])

        bc_l = bc_pool.tile([K, W], f32, tag="bcl")
        bc_r = bc_pool.tile([K, W], f32, tag="bcr")
        nc.tensor.matmul(bc_l[:], ones_sel[:], inv_l[:])
        nc.tensor.matmul(bc_r[:], ones_sel[:], inv_r[:])

        rp = rpads[g % 2]
        nc.vector.tensor_mul(rp[:, D - 1 :], r_raw[:], bc_r[:])

        strip = None
        l_n = None
        if R > 1:
            # Normalize-multiply straight into the strip's diagonal blocks:
            # per hb one DVE mul whose output AP scatters the [C, W] product
            # across the XC chunk positions (128-col steps). No DMAs.
            strip = strips[g % 2]
            for hb in range(R):
                dst0 = strip[hb * C : (hb + 1) * C, hb * cch : hb * cch + cch]
                pstride = int(dst0.ap[0][0])
                dst = bass.AP(
                    dst0.tensor, dst0.offset, [[pstride, C], [128, XC], [1, cch]]
                )
                lr0 = l_raw[hb * C : (hb + 1) * C, :]
                lr = bass.AP(lr0.tensor, lr0.offset, [[W, C], [cch, XC], [1, cch]])
                bc0 = bc_l[hb * C : (hb + 1) * C, :]
                bcs = int(bc0.ap[0][0])
                bc = bass.AP(bc0.tensor, bc0.offset, [[bcs, C], [cch, XC], [1, cch]])
                nc.vector.tensor_mul(dst, lr, bc)
        else:
            l_n = ln_pool.tile([K, W], f32, tag="ln")
            nc.vector.tensor_mul(l_n[:], l_raw[:], bc_l[:])

        QB = min(4, XC)
        band_sb = None
        for xc in range(XC):
            x0 = xc * cch
            if R > 1:
                stat = strip[:, xc * 128 : (xc + 1) * 128]
            else:
                stat = l_n[:]
            band = band_pool.tile([128, B], f32, tag="band")
            nc.tensor.matmul(band[:], stat, rp[:, x0 : x0 + B])
            slot = ci % QB
            if slot == 0:
                band_sb = bout_pool.tile([128, QB * B], f32, tag="bout")
            dst = band_sb[:, slot * B : (slot + 1) * B]
            if ci % 2 == 0:
                nc.scalar.copy(dst, band[:])
            else:
                nc.vector.tensor_copy(dst, band[:])
            if slot == QB - 1:
                nc.sync.dma_start(out_dram[ci // QB], band_sb[:])
            ci += 1


def _build_nc():
    if "nc" in _NC_CACHE:
        return _NC_CACHE["nc"]
    from contextlib import ExitStack

    import concourse.tile as tile
    from concourse import bacc, mybir

    f32 = mybir.dt.float32
    nc = bacc.Bacc("TRN2", target_bir_lowering=False, debug=False)

    tensors = []
    for i, s in enumerate(SCALES):
        C, H, W, D, R, Hl, K, cch, B, G, XC = _derived(s)
        l = nc.dram_tensor(f"left{i}", [2, C, Hl, W], f32, kind="ExternalInput")
        r = nc.dram_tensor(f"right{i}", [2, C, Hl, W], f32, kind="ExternalInput")
        QB = min(4, XC)
        o = nc.dram_tensor(
            f"out{i}", [G * XC // QB, 128, QB * B], f32, kind="ExternalOutput"
        )
        tensors.append((l, r, o))

    with tile.TileContext(nc) as tc:
        with ExitStack() as ctx:
            ec = ctx.enter_context
            pools = {
                "io": ec(tc.tile_pool(name="io", bufs=3)),
                "sq": ec(tc.tile_pool(name="sq", bufs=2)),
                "small": ec(tc.tile_pool(name="small", bufs=2)),
                "ln": ec(tc.tile_pool(name="ln", bufs=2)),
                "pers": ec(tc.tile_pool(name="pers", bufs=1)),
                "rpad": ec(tc.tile_pool(name="rpad", bufs=1)),
                "bd": ec(tc.tile_pool(name="bd", bufs=1)),
                "bout": ec(tc.tile_pool(name="bout", bufs=4)),
                # PSUM: np (2 tags) + bc (2 tags) + band (4) = 8 banks,
                # shared by all scales (slots sized to the scale0 max).
                "npsum": ec(tc.tile_pool(name="npsum", bufs=1, space="PSUM")),
                "bc": ec(tc.tile_pool(name="bc", bufs=1, space="PSUM")),
                "band": ec(tc.tile_pool(name="band", bufs=4, space="PSUM")),
            }
            for i, s in enumerate(SCALES):
                l, r, o = tensors[i]
                _emit_scale(nc, tc, pools, i, s, l, r, o)
    nc.compile()
    _NC_CACHE["nc"] = nc
    return nc


def _assemble(arr, s):
    """[G*XC/QB, 128, QB*B] batched bands -> [2, D, Hl, W] block for one core."""
    C, H, W, D, R, Hl, K, cch, B, G, XC = _derived(s)
    QB = min(4, XC)
    # unpack store batches: [NS, 128, QB, B] -> chunk-major [G*XC, 128, B]
    arr = np.ascontiguousarray(arr).reshape(G, XC // QB, 128, QB, B)
    arr = arr.transpose(0, 1, 3, 2, 4)
    arr = np.ascontiguousarray(arr).reshape(G, XC, R, cch, B)
    e = arr.strides[-1]
    ext = np.lib.stride_tricks.as_strided(
        arr,
        shape=(G, XC, R, cch, D),
        strides=(
            arr.strides[0],
            arr.strides[1],
            arr.strides[2],
            arr.strides[3] + e,  # x' advances one row AND one column: diagonal
            e,
        ),
    )
    # ext[g, xc, hb, x', v] = cost[n, D-1-v, h_local, xc*cch + x']
    out = ext.transpose(0, 2, 4, 1, 3).reshape(2, Hl, D, W)
    out = out.transpose(0, 2, 1, 3)[:, ::-1, :, :]
    return np.ascontiguousarray(out)


def _get_runner():
    """Build (once) a reusable jitted SPMD runner over the 8 cores.

    Mirrors concourse.bass2jax.run_bass_via_pjrt, but: (a) the jitted
    callable is cached so repeat calls don't re-trace/re-compile, and
    (b) the NEFF's output buffers are jnp.zeros created inside the jit
    (every output element is written by the kernel, so their initial
    value is irrelevant) — no per-call host->device output transfer.
    """
    if "runner" in _NC_CACHE:
        return _NC_CACHE["runner"]

    import jax
    import jax.numpy as jnp
    from jax.sharding import Mesh, PartitionSpec
    from jax.experimental.shard_map import shard_map

    from concourse import bass2jax, mybir

    nc = _build_nc()
    bass2jax.install_neuronx_cc_hook()

    partition_name = nc.partition_id_tensor.name if nc.partition_id_tensor else None
    in_names, out_names, out_avals = [], [], []
    for alloc in nc.m.functions[0].allocations:
        if not isinstance(alloc, mybir.MemoryLocationSet):
            continue
        name = alloc.memorylocations[0].name
        if alloc.kind == "ExternalInput":
            if name != partition_name:
                in_names.append(name)
        elif alloc.kind == "ExternalOutput":
            shape = tuple(alloc.tensor_shape)
            dtype = mybir.dt.np(alloc.dtype)
            out_avals.append(jax.core.ShapedArray(shape, dtype))
            out_names.append(name)
    n_params = len(in_names)
    all_in_names = list(in_names) + list(out_names)
    if partition_name is not None:
        all_in_names.append(partition_name)

    def _body(*args):
        operands = list(args)
        if partition_name is not None:
            operands.append(bass2jax.partition_id_tensor())
        outs = bass2jax._bass_exec_p.bind(
            *operands,
            out_avals=tuple(out_avals),
            in_names=tuple(all_in_names),
            out_names=tuple(out_names),
            lowering_input_output_aliases=(),
            sim_require_finite=True,
            sim_require_nnan=True,
            nc=nc,
        )
        return tuple(outs)

    devices = jax.devices()[:N_CORES]
    mesh = Mesh(np.asarray(devices), ("core",))
    sharded = jax.jit(
        shard_map(
            _body,
            mesh=mesh,
            in_specs=(PartitionSpec("core"),) * (n_params + len(out_names)),
            out_specs=(PartitionSpec("core"),) * len(out_names),
            check_rep=False,
        ),
        keep_unused=True,
    )
    runner = dict(
        nc=nc,
        sharded=sharded,
        in_names=in_names,
        out_names=out_names,
        out_avals=out_avals,
        mesh=mesh,
    )
    _NC_CACHE["runner"] = runner
    return runner


def _shard_inputs(lefts, rights):
    """Full inputs -> concat-along-axis0 per-tensor arrays for shard_map."""
    concat = {}
    for i, s in enumerate(SCALES):
        Hl = s["H"] // N_CORES
        for nm, src in ((f"left{i}", lefts[i]), (f"right{i}", rights[i])):
            parts = [
                np.ascontiguousarray(src[:, :, k * Hl : (k + 1) * Hl, :], np.float32)
                for k in range(N_CORES)
            ]
            concat[nm] = np.concatenate(parts, axis=0)
    return concat


def _zero_args(r):
    return [
        np.zeros((N_CORES * a.shape[0], *a.shape[1:]), a.dtype)
        for a in r["out_avals"]
    ]


def _run(concat_inputs):
    r = _get_runner()
    args = [concat_inputs[nm] for nm in r["in_names"]] + _zero_args(r)
    out_arrs = r["sharded"](*args)
    res = []
    for i, nm in enumerate(r["out_names"]):
        a = np.asarray(out_arrs[i])
        res.append(a.reshape(N_CORES, *r["out_avals"][i].shape))
    return dict(zip(r["out_names"], res))


def kernel(left0, right0, left1, right1, left2, right2, max_disparity):
    lefts = [np.asarray(left0), np.asarray(left1), np.asarray(left2)]
    rights = [np.asarray(right0), np.asarray(right1), np.asarray(right2)]

    concat_inputs = _shard_inputs(lefts, rights)
    res = _run(concat_inputs)

    outs = []
    for i, s in enumerate(SCALES):
        blocks = [_assemble(res[f"out{i}"][k], s) for k in range(N_CORES)]
        outs.append(np.concatenate(blocks, axis=2))
    return tuple(outs)


def bench(iters=5):
    """Time repeated device executions with device-resident inputs."""
    import time

    import jax

    rng = np.random.default_rng(0)
    lefts, rights = [], []
    for s in SCALES:
        lefts.append(rng.standard_normal((2, s["C"], s["H"], s["W"])).astype(np.float32))
        rights.append(rng.standard_normal((2, s["C"], s["H"], s["W"])).astype(np.float32))
    concat_inputs = _shard_inputs(lefts, rights)
    r = _get_runner()
    from jax.sharding import NamedSharding, PartitionSpec

    sh = NamedSharding(r["mesh"], PartitionSpec("core"))
    args = [
        jax.device_put(a, sh)
        for a in [concat_inputs[nm] for nm in r["in_names"]] + _zero_args(r)
    ]
    times = []
    for it in range(iters + 1):
        t0 = time.perf_counter()
        out = r["sharded"](*args)
        jax.block_until_ready(out)
        dt = time.perf_counter() - t0
        if it > 0:
            times.append(dt)
    return times
